# revision 1
# baseline (speedup 1.0000x reference)
"""Bass/Trainium2 kernel for batched 3D FFT circular convolution.

Reference computes: y = Re(IFFT3(FFT3(x) . FFT3(w))) / (N * sqrt(N)) scaling
(net: y = circular_conv3d(x, w) / sqrt(N)), x: (16, 32, 128, 128) f32,
w: (32, 128, 128) f32.

Strategy (pure data parallel over batch, 8 cores x 2 samples):
- Pack two real samples as one complex volume z = x0 + i*x1. Then
  y_pair = IFFT3(FFT3(z) * W~) and y0 = Re, y1 = Im (exact because w real).
- FFTs as DFT-matrix matmuls on the tensor engine (fp32r, full rate at N>=512),
  transform axis on partitions; PE transposes between axes; the size-32 axis is
  transformed with a block-diagonal 4x(32x32) DFT so the full 128-partition
  contraction stays busy.
- W~ = FFT3(w) / (N*sqrt(N)) computed on-device per core (replicated).

Layouts per stage (partition | free), free index given as linear combination:
  L0  [d2 | d1,d3]   f = d1*128 + d3          (natural DMA: 512B runs)
  A   FFT d2   -> [k2 | d1,d3]
  T1  per-d1 transpose -> [d3 | d1,k2]
  B   FFT d3   -> evict-scatter -> [k3 | k2l,k2h,d1]  f = k2l*128 + k2h*32 + d1
  T2  per-k2l transpose -> [(k2h,d1) | k2l,k3]
  C   FFT d1 (block-diag) -> [(k2h,k1) | k2l,k3];  * W~ fused into eviction
  D   IFFT d1  -> [(k2h,d1') | k2l,k3]
  T3  per-k2l transpose, evict-scatter -> [k3 | d1',k2]  f = d1'*128 + k2
  E   IFFT d3  -> [d3' | d1',k2]
  T4  per-d1' transpose -> [k2 | d1',d3']
  F   IFFT d2  -> [d2' | d1',d3'] -> DMA out
"""

import numpy as np

D1, D2, D3 = 32, 128, 128
NTOT = D1 * D2 * D3
FREE = D1 * D3  # 4096
B = 16
NCORES = 8

# const matrix slots in the packed (10,128,128) consts input
F2R, F2I, F2In, F2Rs, F2Is, BDR, BDI, BDIn, IDENT, _PAD = range(10)


def _tf32(a):
    """Round fp32 array to tf32 (10-bit mantissa, round-to-nearest-even)."""
    b = np.ascontiguousarray(a, dtype=np.float32).view(np.uint32)
    r = b + np.uint32(0x00000FFF) + ((b >> np.uint32(13)) & np.uint32(1))
    r &= np.uint32(0xFFFFE000)
    return r.view(np.float32)


def _consts_np():
    k = np.arange(128)
    F2 = np.exp(-2j * np.pi * np.outer(k, k) / 128)
    k1 = np.arange(32)
    F1 = np.exp(-2j * np.pi * np.outer(k1, k1) / 32)
    BD = np.zeros((128, 128), complex)
    for g in range(4):
        BD[32 * g:32 * g + 32, 32 * g:32 * g + 32] = F1
    alpha = 1.0 / (NTOT * np.sqrt(np.float32(NTOT), dtype=np.float64))
    mats = np.stack([
        F2.real, F2.imag, -F2.imag,
        F2.real * alpha, F2.imag * alpha,
        BD.real, BD.imag, -BD.imag,
        np.eye(128), np.zeros((128, 128)),
    ])
    return _tf32(np.ascontiguousarray(mats, dtype=np.float32))


def _build_program():
    import concourse.mybir as mybir
    import concourse.tile as tile
    from concourse import bacc

    f32 = mybir.dt.float32
    f32r = mybir.dt.float32r

    nc = bacc.Bacc("TRN2")
    x0_d = nc.dram_tensor("x0", (D1, D2, D3), f32r, kind="ExternalInput")
    x1_d = nc.dram_tensor("x1", (D1, D2, D3), f32r, kind="ExternalInput")
    w_d = nc.dram_tensor("w", (D1, D2, D3), f32r, kind="ExternalInput")
    c_d = nc.dram_tensor("consts", (10, 128, 128), f32r, kind="ExternalInput")
    y0_d = nc.dram_tensor("y0", (D1, D2, D3), f32, kind="ExternalOutput")
    y1_d = nc.dram_tensor("y1", (D1, D2, D3), f32, kind="ExternalOutput")

    with tile.TileContext(nc) as tc:
        with (
            tc.tile_pool(name="sb", bufs=1) as sb,
            tc.tile_pool(name="tp", bufs=3) as tp,
            tc.tile_pool(name="ps", bufs=4, space="PSUM") as ps,
        ):
            consts = sb.tile([128, 10 * 128], f32r, name="consts")
            nc.sync.dma_start(
                out=consts.rearrange("p (n f) -> p n f", n=10),
                in_=c_d.ap().rearrange("n p f -> p n f"))

            def M(i):
                return consts[:, i * 128:(i + 1) * 128]

            zA = [sb.tile([128, FREE], f32r, name=f"zA{c}") for c in range(2)]
            zB = [sb.tile([128, FREE], f32r, name=f"zB{c}") for c in range(2)]
            wA = [sb.tile([128, FREE], f32r, name=f"wA{c}") for c in range(2)]
            wB = [sb.tile([128, FREE], f32r, name=f"wB{c}") for c in range(2)]
            wN = sb.tile([128, FREE], f32, name="wN")  # -W~I

            # input DMAs (natural layout: partition = d2, 512B runs along d3)
            def load3(dst, src_d):
                nc.sync.dma_start(
                    out=dst.rearrange("p (a c) -> p a c", a=D1),
                    in_=src_d.ap().rearrange("a b c -> b a c"))

            load3(wA[0], w_d)
            load3(zA[0], x0_d)
            load3(zA[1], x1_d)

            ectr = [0]

            def evict(dst, src):
                # alternate psum->sbuf eviction between DVE and ACT
                if ectr[0] % 2 == 0:
                    nc.vector.tensor_copy(dst, src)
                else:
                    nc.scalar.copy(dst, src)
                ectr[0] += 1

            def scatter_dst(buf, kind, t):
                if kind == "B":  # psum enum (d1 8, k2h 4, k2l 32) -> f=k2l*128+k2h*32+d1
                    v = buf.rearrange("p (k2l k2h d1) -> p d1 k2h k2l",
                                      k2l=32, k2h=4, d1=32)
                    return v[:, 8 * t:8 * (t + 1), :, :]
                else:  # T3: psum enum (k2l 8, k2h 4, d1 32) -> f=d1*128+k2h*32+k2l
                    v = buf.rearrange("p (d1 k2h k2l) -> p k2l k2h d1",
                                      d1=32, k2h=4, k2l=32)
                    return v[:, 8 * t:8 * (t + 1), :, :]

            def fft_stage(dst, src, mR, mI, mIn, scatter=None, mid=None,
                          out_f32=False):
                """out_R = mR^T R + mIn^T I ; out_I = mI^T R + mR^T I."""
                for t in range(4):
                    pR = ps.tile([128, 1024], f32, name="pR", tag="ps")
                    pI = ps.tile([128, 1024], f32, name="pI", tag="ps")
                    for h in range(2):
                        s = slice(1024 * t + 512 * h, 1024 * t + 512 * (h + 1))
                        o = slice(512 * h, 512 * (h + 1))
                        rhs = src[0][:, s]
                        rhsI = src[1][:, s]
                        nc.tensor.matmul(pR[:, o], M(mR), rhs,
                                         start=True, stop=False)
                        nc.tensor.matmul(pI[:, o], M(mI), rhs,
                                         start=True, stop=False)
                        nc.tensor.matmul(pR[:, o], M(mIn), rhsI,
                                         start=False, stop=True)
                        nc.tensor.matmul(pI[:, o], M(mR), rhsI,
                                         start=False, stop=True)
                    sl = slice(1024 * t, 1024 * (t + 1))
                    if mid is not None:
                        # fused pointwise: V = Z * W~ straight out of PSUM
                        wR, wI, wIn = mid
                        t1 = tp.tile([128, 1024], f32, name="t1", tag="t1")
                        t2 = tp.tile([128, 1024], f32, name="t2", tag="t2")
                        nc.vector.tensor_tensor(t1, pR, wR[:, sl],
                                                op=mybir.AluOpType.mult)
                        nc.vector.tensor_tensor(t2, pI, wIn[:, sl],
                                                op=mybir.AluOpType.mult)
                        nc.vector.tensor_add(dst[0][:, sl], t1, t2)
                        t3 = tp.tile([128, 1024], f32, name="t3", tag="t1")
                        t4 = tp.tile([128, 1024], f32, name="t4", tag="t2")
                        nc.vector.tensor_tensor(t3, pR, wI[:, sl],
                                                op=mybir.AluOpType.mult)
                        nc.vector.tensor_tensor(t4, pI, wR[:, sl],
                                                op=mybir.AluOpType.mult)
                        nc.vector.tensor_add(dst[1][:, sl], t3, t4)
                    elif scatter is not None:
                        nc.vector.tensor_copy(
                            scatter_dst(dst[0], scatter, t),
                            pR.rearrange("p (a b c) -> p a b c", a=8, b=4, c=32))
                        nc.scalar.copy(
                            scatter_dst(dst[1], scatter, t),
                            pI.rearrange("p (a b c) -> p a b c", a=8, b=4, c=32))
                    else:
                        dR = dst[0][:, sl]
                        dI = dst[1][:, sl]
                        if out_f32:
                            dR = dR.bitcast(f32)
                            dI = dI.bitcast(f32)
                        evict(dR, pR)
                        evict(dI, pI)

            def fft_stage_real(dst, src, mR, mI):
                # real input: out_R = mR^T R ; out_I = mI^T R
                for t in range(4):
                    pR = ps.tile([128, 1024], f32, name="pR", tag="ps")
                    pI = ps.tile([128, 1024], f32, name="pI", tag="ps")
                    for h in range(2):
                        s = slice(1024 * t + 512 * h, 1024 * t + 512 * (h + 1))
                        o = slice(512 * h, 512 * (h + 1))
                        rhs = src[:, s]
                        nc.tensor.matmul(pR[:, o], M(mR), rhs)
                        nc.tensor.matmul(pI[:, o], M(mI), rhs)
                    sl = slice(1024 * t, 1024 * (t + 1))
                    evict(dst[0][:, sl], pR)
                    evict(dst[1][:, sl], pI)

            def t_stage(dst, src, scatter=None):
                """per-128-block PE transposes. src/dst are [R, I] pairs."""
                for comp in range(2):
                    for g in range(4):
                        pT = ps.tile([128, 1024], f32r, name="pT", tag="ps")
                        for j in range(8):
                            blk = g * 8 + j
                            nc.tensor.transpose(
                                pT[:, 128 * j:128 * (j + 1)],
                                src[comp][:, 128 * blk:128 * (blk + 1)],
                                M(IDENT))
                        sl = slice(1024 * g, 1024 * (g + 1))
                        if scatter is not None:
                            if (comp + g) % 2 == 0:
                                nc.vector.tensor_copy(
                                    scatter_dst(dst[comp], scatter, g),
                                    pT.rearrange("p (a b c) -> p a b c",
                                                 a=8, b=4, c=32))
                            else:
                                nc.scalar.copy(
                                    scatter_dst(dst[comp], scatter, g),
                                    pT.rearrange("p (a b c) -> p a b c",
                                                 a=8, b=4, c=32))
                        else:
                            evict(dst[comp][:, sl], pT)

            # ---------------- W chain ----------------
            fft_stage_real((wB[0], wB[1]), wA[0], F2Rs, F2Is)        # [k2|d1,d3]
            t_stage(wA, wB)                                          # [d3|d1,k2]
            fft_stage(wB, wA, F2R, F2I, F2In, scatter="B")           # [k3|k2l,k2h,d1]
            t_stage(wA, wB, scatter=None)                            # [(k2h,d1)|k2l,k3]
            fft_stage(wB, wA, BDR, BDI, BDIn)                        # W~ in wB
            # wN = -W~I
            for t in range(4):
                sl = slice(1024 * t, 1024 * (t + 1))
                nc.vector.tensor_scalar_mul(wN[:, sl], wB[1][:, sl], -1.0)

            # ---------------- z chain ----------------
            fft_stage(zB, zA, F2R, F2I, F2In)                        # A
            t_stage(zA, zB)                                          # T1
            fft_stage(zB, zA, F2R, F2I, F2In, scatter="B")           # B
            t_stage(zA, zB)                                          # T2
            fft_stage(zB, zA, BDR, BDI, BDIn,
                      mid=(wB[0], wB[1], wN))                        # C + pointwise
            fft_stage(zA, zB, BDR, BDIn, BDI)                        # D (inverse)
            t_stage(zB, zA, scatter="T3")                            # T3
            fft_stage(zA, zB, F2R, F2In, F2I)                        # E (inverse)
            t_stage(zB, zA)                                          # T4
            fft_stage(zA, zB, F2R, F2In, F2I)                        # F (inverse)

            # outputs
            nc.sync.dma_start(
                out=y0_d.ap().rearrange("a b c -> b a c"),
                in_=zA[0].bitcast(f32).rearrange("p (a c) -> p a c", a=D1))
            nc.sync.dma_start(
                out=y1_d.ap().rearrange("a b c -> b a c"),
                in_=zA[1].bitcast(f32).rearrange("p (a c) -> p a c", a=D1))
    return nc


_CACHE = {}


def _get_program():
    if "nc" not in _CACHE:
        nc = _build_program()
        try:
            if not nc.is_finalized():
                nc.finalize()
        except AttributeError:
            nc.finalize()
        _CACHE["nc"] = nc
    return _CACHE["nc"]


def _run(x, w_real, **kw):
    from concourse.bass_utils import run_bass_kernel_spmd

    nc = _get_program()
    consts = _consts_np()
    x = _tf32(np.ascontiguousarray(x, dtype=np.float32))
    w = _tf32(np.ascontiguousarray(w_real, dtype=np.float32))
    in_maps = []
    for c in range(NCORES):
        in_maps.append({
            "x0": x[2 * c],
            "x1": x[2 * c + 1],
            "w": w,
            "consts": consts,
        })
    res = run_bass_kernel_spmd(nc, in_maps, core_ids=list(range(NCORES)), **kw)
    out = np.empty((B, D1, D2, D3), dtype=np.float32)
    for c in range(NCORES):
        out[2 * c] = res.results[c]["y0"]
        out[2 * c + 1] = res.results[c]["y1"]
    return out, res


def kernel(x: np.ndarray, w_real: np.ndarray) -> np.ndarray:
    return _run(x, w_real)[0]


def kernel_traced(x: np.ndarray, w_real: np.ndarray):
    return _run(x, w_real, trace=True)



# revision 3
# speedup vs baseline: 1.2910x; 1.2910x over previous
"""Bass/Trainium2 kernel for batched 3D FFT circular convolution.

Reference computes y = Re(IFFT3(FFT3(x) * FFT3(w))) with net scaling
circular_conv3d(x, w) / sqrt(N); x: (16, 32, 128, 128) f32, w: (32, 128, 128).

Strategy (pure data parallel over batch, 8 cores x 2 samples):
- Pack two real samples as one complex volume z = x0 + i*x1; y0 = Re, y1 = Im.
- All FFTs as DFT matmuls in bf16 (inputs pre-rounded to bf16 on host).
- Transposes are FUSED into the DFT matmuls: the DFT matrix is symmetric, so
  making the DATA the stationary operand computes data^T @ F = (F @ data)^T --
  the transform output lands transposed (next axis on partitions) for free.
  bf16 enables fast-weight-load, so the per-block stationary reloads are cheap.
- The size-32 axis uses a block-diagonal 4x(32x32) DFT at full 128 contraction.
- W~ = FFT3(w)/(N*sqrt(N)) computed on-device per core (replicated).

Per-stage layouts (partition | free):
  L0   [d2 | d1,d3]                     f = d1*128 + d3      (natural DMA)
  S1   fused FFT d2  -> [d3 | j,k2q,d1] f = j*128+k2q*32+d1  (k2 = 4j+k2q)
  S2   fused FFT d3  -> [(k2q,d1) | j,k3]  f = j*128 + k3
  S3   BD FFT d1 (weight-stationary) -> [(k2q,k1) | j,k3]
  M    V = Z * W~   (DVE, bf16 2x mode)
  S4   fused BD IFFT d1 -> [k3 | d1,k2]  f = d1*128 + k2
  S5   fused IFFT k3 -> [k2 | d1,d3]     f = d1*128 + d3
  S6   IFFT k2 (weight-stationary) -> [d2 | d1,d3] -> DMA out
"""

import numpy as np
import ml_dtypes

BF = ml_dtypes.bfloat16

D1, D2, D3 = 32, 128, 128
NTOT = D1 * D2 * D3
FREE = D1 * D3  # 4096
B = 16
NCORES = 8

# const matrix slots in the packed (8,128,128) consts input
F2R, F2I, F2In, F2Rs, F2Is, BDR, BDI, BDIn = range(8)
NCONST = 8


def _consts_np():
    k = np.arange(128)
    F2 = np.exp(-2j * np.pi * np.outer(k, k) / 128)
    k1 = np.arange(32)
    F1 = np.exp(-2j * np.pi * np.outer(k1, k1) / 32)
    BD = np.zeros((128, 128), complex)
    for g in range(4):
        BD[32 * g:32 * g + 32, 32 * g:32 * g + 32] = F1
    alpha = 1.0 / (NTOT * np.sqrt(np.float64(NTOT)))
    mats = np.stack([
        F2.real, F2.imag, -F2.imag,
        F2.real * alpha, F2.imag * alpha,
        BD.real, BD.imag, -BD.imag,
    ])
    return np.ascontiguousarray(mats, dtype=np.float32).astype(BF)


def _build_program():
    import concourse.mybir as mybir
    import concourse.tile as tile
    from concourse import bacc

    f32 = mybir.dt.float32
    bf16 = mybir.dt.bfloat16

    nc = bacc.Bacc("TRN2")
    x0_d = nc.dram_tensor("x0", (D1, D2, D3), bf16, kind="ExternalInput")
    x1_d = nc.dram_tensor("x1", (D1, D2, D3), bf16, kind="ExternalInput")
    w_d = nc.dram_tensor("w", (D1, D2, D3), bf16, kind="ExternalInput")
    c_d = nc.dram_tensor("consts", (NCONST, 128, 128), bf16,
                         kind="ExternalInput")
    y0_d = nc.dram_tensor("y0", (D1, D2, D3), f32, kind="ExternalOutput")
    y1_d = nc.dram_tensor("y1", (D1, D2, D3), f32, kind="ExternalOutput")

    with tile.TileContext(nc) as tc:
        with (
            tc.tile_pool(name="sb", bufs=1) as sb,
            tc.tile_pool(name="tp", bufs=2) as tp,
            tc.tile_pool(name="ps", bufs=4, space="PSUM") as ps,
        ):
            consts = sb.tile([128, NCONST * 128], bf16, name="consts")
            nc.sync.dma_start(
                out=consts.rearrange("p (n f) -> p n f", n=NCONST),
                in_=c_d.ap().rearrange("n p f -> p n f"))

            def M(i):
                return consts[:, i * 128:(i + 1) * 128]

            def vol(name, n=2, dt=bf16):
                return [sb.tile([128, FREE], dt, name=f"{name}{c}")
                        for c in range(n)]

            zA = vol("zA")
            zB = vol("zB")
            VV = vol("VV")
            wA = vol("wA", 1)
            wB = vol("wB")
            wC = vol("wC")
            WT = vol("WT")
            yst = vol("yst", 2, f32)

            # input DMAs; x in 4 chunks per component for early compute start
            nc.sync.dma_start(
                out=wA[0].rearrange("p (a c) -> p a c", a=D1),
                in_=w_d.ap().rearrange("a b c -> b a c"))
            for t in range(4):
                for comp, src in ((0, x0_d), (1, x1_d)):
                    nc.sync.dma_start(
                        out=zA[comp][:, 1024 * t:1024 * (t + 1)].rearrange(
                            "p (a c) -> p a c", a=8),
                        in_=src.ap().rearrange("a b c -> b a c")[
                            :, 8 * t:8 * (t + 1), :])

            ectr = [0]

            def evict(dst, src):
                if ectr[0] % 2 == 0:
                    nc.vector.tensor_copy(dst, src)
                else:
                    nc.scalar.copy(dst, src)
                ectr[0] += 1

            def scatter_view(buf, kind, g):
                if kind == "S1":
                    # psum (q, j, k2q) -> f = j*128 + k2q*32 + (8g+q)
                    v = buf.rearrange("p (j k2q d1) -> p d1 j k2q",
                                      j=32, k2q=4, d1=32)
                else:
                    # S4: psum (q, k2q, d1) -> f = d1*128 + (8g+q)*4 + k2q
                    v = buf.rearrange("p (d1 j k2q) -> p j k2q d1",
                                      d1=32, j=32, k2q=4)
                return v[:, 8 * g:8 * (g + 1), :, :]

            def psum_view(buf, kind):
                if kind == "S1":
                    return buf.rearrange("p (q j k2q) -> p q j k2q",
                                         q=8, j=32, k2q=4)
                return buf.rearrange("p (q k2q d1) -> p q k2q d1",
                                     q=8, k2q=4, d1=32)

            def fused_stage(dsts, srcs, mats, scatter=None, real_in=False):
                """data-stationary transform; contracts the partition axis and
                transposes lhsT's free axis onto the output partitions.
                pR = sR^T M(mA) + sI^T M(mB); pI = sR^T M(mC) + sI^T M(mA)."""
                mA, mB, mC = mats
                for g in range(4):
                    pR = ps.tile([128, 1024], f32, name="pR", tag="ps")
                    pI = ps.tile([128, 1024], f32, name="pI", tag="ps")
                    for q in range(8):
                        b = 8 * g + q
                        o = slice(128 * q, 128 * (q + 1))
                        s = slice(128 * b, 128 * (b + 1))
                        # PSUM start=True clears has_written for the WHOLE
                        # bank (HW-verified), so only the first matmul
                        # touching each bank may carry it; later slices rely
                        # on overwrite-where-bit-clear, then accumulate.
                        st = (q % 4 == 0)
                        sp = (q % 4 == 3)
                        if real_in:
                            nc.tensor.matmul(pR[:, o], srcs[0][:, s], M(mA),
                                             start=st, stop=sp,
                                             skip_group_check=True)
                            nc.tensor.matmul(pI[:, o], srcs[0][:, s], M(mC),
                                             start=st, stop=sp,
                                             skip_group_check=True)
                        else:
                            nc.tensor.matmul(pR[:, o], srcs[0][:, s], M(mA),
                                             start=st, stop=False,
                                             skip_group_check=True)
                            nc.tensor.matmul(pI[:, o], srcs[0][:, s], M(mC),
                                             start=st, stop=False,
                                             skip_group_check=True)
                            nc.tensor.matmul(pR[:, o], srcs[1][:, s], M(mB),
                                             start=False, stop=sp,
                                             skip_group_check=True)
                            nc.tensor.matmul(pI[:, o], srcs[1][:, s], M(mA),
                                             start=False, stop=sp,
                                             skip_group_check=True)
                    if scatter is None:
                        sl = slice(1024 * g, 1024 * (g + 1))
                        evict(dsts[0][:, sl], pR)
                        evict(dsts[1][:, sl], pI)
                    else:
                        evict(scatter_view(dsts[0], scatter, g),
                              psum_view(pR, scatter))
                        evict(scatter_view(dsts[1], scatter, g),
                              psum_view(pI, scatter))

            def std_stage(dsts, srcs, mats, out_f32=False):
                """weight-stationary transform along the partition axis.
                pR = M(mA)^T sR + M(mB)^T sI; pI = M(mC)^T sR + M(mA)^T sI."""
                mA, mB, mC = mats
                for t in range(4):
                    pR = ps.tile([128, 1024], f32, name="pR", tag="ps")
                    pI = ps.tile([128, 1024], f32, name="pI", tag="ps")
                    for h in range(2):
                        s = slice(1024 * t + 512 * h, 1024 * t + 512 * (h + 1))
                        o = slice(512 * h, 512 * (h + 1))
                        nc.tensor.matmul(pR[:, o], M(mA), srcs[0][:, s],
                                         start=True, stop=False)
                        nc.tensor.matmul(pI[:, o], M(mC), srcs[0][:, s],
                                         start=True, stop=False)
                        nc.tensor.matmul(pR[:, o], M(mB), srcs[1][:, s],
                                         start=False, stop=True)
                        nc.tensor.matmul(pI[:, o], M(mA), srcs[1][:, s],
                                         start=False, stop=True)
                    sl = slice(1024 * t, 1024 * (t + 1))
                    evict(dsts[0][:, sl], pR)
                    evict(dsts[1][:, sl], pI)

            FWD_F2 = (F2R, F2In, F2I)
            INV_F2 = (F2R, F2I, F2In)
            FWD_BD = (BDR, BDIn, BDI)
            INV_BD = (BDR, BDI, BDIn)

            # interleave W-chain and z-chain stages so stage-boundary stalls
            # of one chain fill with PE work from the other
            fused_stage(wB, wA, (F2Rs, None, F2Is), scatter="S1",
                        real_in=True)                      # S1w
            fused_stage(zB, zA, FWD_F2, scatter="S1")      # S1
            fused_stage(wC, wB, FWD_F2)                    # S2w
            fused_stage(zA, zB, FWD_F2)                    # S2
            std_stage(WT, wC, FWD_BD)                      # S3w
            std_stage(zB, zA, FWD_BD)                      # S3

            # M: V = Z * W~  (bf16 SBUF tensor_tensor, 2x mode)
            for hh in range(2):
                s = slice(2048 * hh, 2048 * (hh + 1))
                t1 = tp.tile([128, 2048], bf16, name="t1", tag="t1")
                t2 = tp.tile([128, 2048], bf16, name="t2", tag="t2")
                nc.vector.tensor_tensor(t1, zB[0][:, s], WT[0][:, s],
                                        op=mybir.AluOpType.mult)
                nc.vector.tensor_tensor(t2, zB[1][:, s], WT[1][:, s],
                                        op=mybir.AluOpType.mult)
                nc.vector.tensor_tensor(VV[0][:, s], t1, t2,
                                        op=mybir.AluOpType.subtract)
                t3 = tp.tile([128, 2048], bf16, name="t3", tag="t1")
                t4 = tp.tile([128, 2048], bf16, name="t4", tag="t2")
                nc.vector.tensor_tensor(t3, zB[0][:, s], WT[1][:, s],
                                        op=mybir.AluOpType.mult)
                nc.vector.tensor_tensor(t4, zB[1][:, s], WT[0][:, s],
                                        op=mybir.AluOpType.mult)
                nc.vector.tensor_tensor(VV[1][:, s], t3, t4,
                                        op=mybir.AluOpType.add)

            fused_stage(zA, VV, INV_BD, scatter="S4")      # S4
            fused_stage(zB, zA, INV_F2)                    # S5
            std_stage(yst, zB, INV_F2)                     # S6 (f32 evict)

            # outputs, per 1024-col chunk
            for t in range(4):
                for comp, dst in ((0, y0_d), (1, y1_d)):
                    nc.sync.dma_start(
                        out=dst.ap().rearrange("a b c -> b a c")[
                            :, 8 * t:8 * (t + 1), :],
                        in_=yst[comp][:, 1024 * t:1024 * (t + 1)].rearrange(
                            "p (a c) -> p a c", a=8))
    return nc


_CACHE = {}


def _get_program():
    if "nc" not in _CACHE:
        nc = _build_program()
        try:
            if not nc.is_finalized():
                nc.finalize()
        except AttributeError:
            nc.finalize()
        _CACHE["nc"] = nc
    return _CACHE["nc"]


def _run(x, w_real, **kw):
    from concourse.bass_utils import run_bass_kernel_spmd

    nc = _get_program()
    consts = _consts_np()
    x = np.ascontiguousarray(x, dtype=np.float32).astype(BF)
    w = np.ascontiguousarray(w_real, dtype=np.float32).astype(BF)
    in_maps = []
    for c in range(NCORES):
        in_maps.append({
            "x0": x[2 * c],
            "x1": x[2 * c + 1],
            "w": w,
            "consts": consts,
        })
    res = run_bass_kernel_spmd(nc, in_maps, core_ids=list(range(NCORES)), **kw)
    out = np.empty((B, D1, D2, D3), dtype=np.float32)
    for c in range(NCORES):
        out[2 * c] = res.results[c]["y0"]
        out[2 * c + 1] = res.results[c]["y1"]
    return out, res


def kernel(x: np.ndarray, w_real: np.ndarray) -> np.ndarray:
    return _run(x, w_real)[0]


def kernel_traced(x: np.ndarray, w_real: np.ndarray):
    return _run(x, w_real, trace=True)


# revision 11
# speedup vs baseline: 1.3416x; 1.0392x over previous
"""Bass/Trainium2 kernel for batched 3D FFT circular convolution.

Reference computes y = Re(IFFT3(FFT3(x) * FFT3(w))) with net scaling
circular_conv3d(x, w) / sqrt(N); x: (16, 32, 128, 128) f32, w: (32, 128, 128).

Strategy (data parallel over batch, 8 cores x 2 samples):
- Pack two real samples as one complex volume z = x0 + i*x1; y0 = Re, y1 = Im.
- All FFTs as DFT matmuls in bf16 (inputs pre-rounded to bf16 on host).
- Transposes are FUSED into the DFT matmuls: the DFT matrix is symmetric, so
  making the DATA the stationary operand computes data^T @ F = (F @ data)^T --
  the transform output lands transposed (next axis on partitions) for free.
- Complex arithmetic pairs both component matrices in ONE 256-wide moving
  operand [F_R | F_I]; each data block needs only 2 matmuls (2 LDW).
- Layout permutations ride in strided stationary APs (single-strided slices,
  plus 32-col tile_position strips for stage S2); PSUM evictions contiguous.
- Host pre-transposes x,w to (d2,d1,d3) and un-transposes y so every DMA run
  is >= 1KB contiguous (descriptor-bound otherwise).
- W~ = FFT3(w)/(N*sqrt(N)) is SHARDED: host modulates w by exp(-2pi*i*16c*
  d2/128) per core (spectral shift), every core runs the same program
  computing k2' in [0,16) of its shifted spectrum = its true k2 shard, then
  an 8-core AllGather assembles the full W~ (overlapped with z compute).

Stage layouts (partition | free):
  L0   [d2 | d1,d3]                  f = d1*128 + d3
  S1   fused FFT d2  -> [d3 | d1,k2] f = d1*128 + k2
  S2   fused FFT d3 (4 col-strips per block j, strided 32-col stationary)
                     -> [(k2q,d1) | j,k3]  f = j*128 + k3   (k2 = 4j+k2q)
  S3   BD FFT d1 (weight-stationary) -> [(k2q,k1) | j,k3]
  M    V = Z * W~   (DVE, bf16 2x mode)
  S4   fused BD IFFT d1 -> [k3 | 32*k2 + d1]
  S5   fused IFFT k3 (single-strided stationary, stride 32)
                     -> [k2 | d1,d3]  f = d1*128 + d3
  S6   IFFT k2 (weight-stationary) -> [d2 | d1,d3] -> DMA out
"""

import numpy as np
import ml_dtypes

BF = ml_dtypes.bfloat16

D1, D2, D3 = 32, 128, 128
NTOT = D1 * D2 * D3
FREE = D1 * D3  # 4096
B = 16
NCORES = 8
SHARD_W = True

# paired (128x256) const slots
PF_F2, PI_F2, PF_F2s, PR_F2, PR_BDq, INV_F2_I, INV_BD_I, PI_F2s = range(8)
NPAIRS_TOT = 8
# single 128-wide consts for weight-stationary stages
S_F2R, S_F2I, S_F2In, S_BDR, S_BDI, S_BDIn = range(6)
NSNG = 6


def _consts_np():
    k = np.arange(128)
    F2 = np.exp(-2j * np.pi * np.outer(k, k) / 128)
    k1 = np.arange(32)
    F1 = np.exp(-2j * np.pi * np.outer(k1, k1) / 32)
    BD = np.zeros((128, 128), complex)
    for g in range(4):
        BD[32 * g:32 * g + 32, 32 * g:32 * g + 32] = F1
    alpha = 1.0 / (NTOT * np.sqrt(np.float64(NTOT)))
    F2R, F2I = F2.real, F2.imag
    BDR, BDI = BD.real, BD.imag
    pairs = [
        np.concatenate([F2R, F2I], axis=1),             # PF_F2   (fwd, sR)
        np.concatenate([-F2I, F2R], axis=1),            # PI_F2   (fwd, sI)
        np.concatenate([F2R * alpha, F2I * alpha], 1),  # PF_F2s  (fwd, sR)
        np.concatenate([F2R, -F2I], axis=1),            # PR_F2   (inv, sR)
        np.concatenate([BDR, -BDI], axis=1),            # PR_BDq  (inv, sR)
        np.concatenate([F2I, F2R], axis=1),             # INV_F2_I (inv, sI)
        np.concatenate([BDI, BDR], axis=1),             # INV_BD_I (inv, sI)
        np.concatenate([-F2I * alpha, F2R * alpha], 1),  # PI_F2s (fwd, sI)
    ]
    singles = [F2R, F2I, -F2I, BDR, BDI, -BDI]
    mats = np.concatenate([np.concatenate(pairs, axis=1),
                           np.concatenate(singles, axis=1)], axis=1)
    return np.ascontiguousarray(mats, dtype=np.float32).astype(BF)


def _build_program():
    import concourse.mybir as mybir
    import concourse.tile as tile
    from concourse import bacc

    f32 = mybir.dt.float32
    bf16 = mybir.dt.bfloat16

    nc = bacc.Bacc("TRN2")
    # inputs pre-transposed on host to (d2, d1, d3)
    x0_d = nc.dram_tensor("x0", (D2, D1, D3), bf16, kind="ExternalInput")
    x1_d = nc.dram_tensor("x1", (D2, D1, D3), bf16, kind="ExternalInput")
    w0_d = nc.dram_tensor("w0", (D2, D1, D3), bf16, kind="ExternalInput")
    w1_d = nc.dram_tensor("w1", (D2, D1, D3), bf16, kind="ExternalInput")
    CW = NPAIRS_TOT * 256 + NSNG * 128
    c_d = nc.dram_tensor("consts", (128, CW), bf16, kind="ExternalInput")
    y0_d = nc.dram_tensor("y0", (D2, D1, D3), f32, kind="ExternalOutput")
    y1_d = nc.dram_tensor("y1", (D2, D1, D3), f32, kind="ExternalOutput")

    with tile.TileContext(nc) as tc:
        with (
            tc.tile_pool(name="sb", bufs=1) as sb,
            tc.tile_pool(name="tp", bufs=2) as tp,
            tc.tile_pool(name="ps", bufs=2, space="PSUM") as ps,
            tc.tile_pool(name="dr", bufs=1, space="DRAM") as dr,
        ):
            consts = sb.tile([128, CW], bf16, name="consts")
            nc.sync.dma_start(out=consts, in_=c_d.ap())

            def P2(i):
                return consts[:, 256 * i:256 * (i + 1)]

            def S1m(i):
                o = NPAIRS_TOT * 256
                return consts[:, o + 128 * i:o + 128 * (i + 1)]

            def vol(name, n=2, dt=bf16, cols=FREE):
                return [sb.tile([128, cols], dt, name=f"{name}{c}")
                        for c in range(n)]

            zA = vol("zA")
            zB = vol("zB")
            VV = vol("VV")
            wA = vol("wA")
            wB = vol("wB")
            WT = vol("WT")
            yst = vol("yst", 2, f32)
            if SHARD_W:
                wC = vol("wC", 2, cols=512)
                WTs = vol("WTs", 2, cols=512)
                wt_in = dr.tile([2, 128, 512], bf16, name="wt_in")
                wt_out = dr.tile([8, 2, 128, 512], bf16, name="wt_out",
                                 addr_space="Shared")
            else:
                wC = vol("wC")

            # input DMAs (w first; contiguous >=2KB runs after host transpose)
            for comp, src in ((0, w0_d), (1, w1_d)):
                nc.sync.dma_start(
                    out=wA[comp].rearrange("p (a c) -> p a c", a=D1),
                    in_=src.ap())
            for t in range(4):
                for comp, src in ((0, x0_d), (1, x1_d)):
                    nc.sync.dma_start(
                        out=zA[comp][:, 1024 * t:1024 * (t + 1)].rearrange(
                            "p (a c) -> p a c", a=8),
                        in_=src.ap()[:, 8 * t:8 * (t + 1), :])

            ectr = [0]

            def evict(dst, src):
                # weighted split: DVE also runs the 18us multiply, so give
                # ACT ~7/12 of the copies
                if (ectr[0] * 5) % 12 < 5:
                    nc.vector.tensor_copy(dst, src)
                else:
                    nc.scalar.copy(dst, src)
                ectr[0] += 1

            def lhs_for(src, b, stat):
                if stat == "contig":
                    return src[:, 128 * b:128 * (b + 1)]
                # "stride32": f = 32*k2 + d1 -> fixed d1=b, k2 stride 32
                v = src.rearrange("p (k2 d1) -> p k2 d1", k2=128, d1=32)
                return v[:, :, b:b + 1]

            def fused_stage(dsts, srcs, pairR, pairI, stat="contig",
                            real_in=False, nblk=32):
                for g in range(nblk // 8 if nblk >= 8 else 1):
                    nq = min(8, nblk)
                    pt = ps.tile([128, 256 * nq], f32, name="pt", tag="ps")
                    for q in range(nq):
                        b = (8 * g + q) if nblk >= 8 else q
                        o = slice(256 * q, 256 * (q + 1))
                        st = (q % 2 == 0)
                        sp = (q % 2 == 1) or (nq < 2)
                        if stat == "strips":
                            v0 = srcs[0].rearrange("p (d1 k2) -> p k2 d1",
                                                   d1=32, k2=128)
                            v1 = srcs[1].rearrange("p (d1 k2) -> p k2 d1",
                                                   d1=32, k2=128)
                            for s in range(4):
                                po = pt[32 * s:32 * (s + 1), o]
                                nc.tensor.matmul(
                                    po, v0[:, 4 * b + s, :], P2(pairR),
                                    start=st, stop=False,
                                    tile_position=(0, 32 * s),
                                    skip_group_check=True)
                                nc.tensor.matmul(
                                    po, v1[:, 4 * b + s, :], P2(pairI),
                                    start=False, stop=sp,
                                    tile_position=(0, 32 * s),
                                    skip_group_check=True)
                            continue
                        if real_in:
                            nc.tensor.matmul(pt[:, o], lhs_for(srcs[0], b, stat),
                                             P2(pairR), start=st, stop=sp,
                                             skip_group_check=True)
                        else:
                            nc.tensor.matmul(pt[:, o], lhs_for(srcs[0], b, stat),
                                             P2(pairR), start=st, stop=False,
                                             skip_group_check=True)
                            nc.tensor.matmul(pt[:, o], lhs_for(srcs[1], b, stat),
                                             P2(pairI), start=False, stop=sp,
                                             skip_group_check=True)
                    pv = pt.rearrange("p (q c f) -> p c q f", q=nq, c=2)
                    sl = slice(128 * nq * g, 128 * nq * (g + 1))
                    dv0 = dsts[0][:, sl].rearrange("p (q f) -> p q f", q=nq)
                    dv1 = dsts[1][:, sl].rearrange("p (q f) -> p q f", q=nq)
                    evict(dv0, pv[:, 0])
                    evict(dv1, pv[:, 1])

            def std_stage(dsts, srcs, mats, nchunk=8):
                mA, mB, mC = mats
                for t in range(0, nchunk, 2):
                    nh = min(2, nchunk - t)
                    w_ = 512 * nh
                    pt = ps.tile([128, 2 * w_], f32, name="pt", tag="ps")
                    for h in range(nh):
                        s = slice(512 * (t + h), 512 * (t + h + 1))
                        oR = slice(512 * h, 512 * (h + 1))
                        oI = slice(w_ + 512 * h, w_ + 512 * (h + 1))
                        nc.tensor.matmul(pt[:, oR], S1m(mA), srcs[0][:, s],
                                         start=True, stop=False)
                        nc.tensor.matmul(pt[:, oI], S1m(mC), srcs[0][:, s],
                                         start=True, stop=False)
                        nc.tensor.matmul(pt[:, oR], S1m(mB), srcs[1][:, s],
                                         start=False, stop=True)
                        nc.tensor.matmul(pt[:, oI], S1m(mA), srcs[1][:, s],
                                         start=False, stop=True)
                    sl = slice(512 * t, 512 * (t + nh))
                    evict(dsts[0][:, sl], pt[:, :w_])
                    evict(dsts[1][:, sl], pt[:, w_:])

            # ---- W chain (sharded) interleaved with z chain ----
            fused_stage(wB, wA, PF_F2s, PI_F2s)                   # S1w
            if SHARD_W:
                fused_stage(wC, wB, PF_F2, PI_F2, stat="strips",
                            nblk=4)                               # S2w shard
                std_stage(WTs, wC, (S_BDR, S_BDIn, S_BDI),
                          nchunk=1)                               # S3w shard
                for comp in range(2):
                    nc.sync.dma_start(out=wt_in[comp], in_=WTs[comp])
                nc.gpsimd.collective_compute(
                    "AllGather", mybir.AluOpType.bypass,
                    replica_groups=[list(range(NCORES))],
                    ins=[wt_in[:].opt()], outs=[wt_out[:].opt()])
                for r in range(NCORES):
                    for comp in range(2):
                        nc.sync.dma_start(
                            out=WT[comp][:, 512 * r:512 * (r + 1)],
                            in_=wt_out[r, comp])
            fused_stage(zB, zA, PF_F2, PI_F2)                     # S1
            fused_stage(zA, zB, PF_F2, PI_F2, stat="strips")      # S2
            if not SHARD_W:
                fused_stage(wC, wB, PF_F2, PI_F2, stat="strips")  # S2w full
                std_stage(WT, wC, (S_BDR, S_BDIn, S_BDI))         # S3w full
            std_stage(zB, zA, (S_BDR, S_BDIn, S_BDI))             # S3

            # M: V = Z * W~  (bf16 SBUF tensor_tensor, 2x mode)
            for hh in range(2):
                s = slice(2048 * hh, 2048 * (hh + 1))
                t1 = tp.tile([128, 2048], bf16, name="t1", tag="t1")
                t2 = tp.tile([128, 2048], bf16, name="t2", tag="t2")
                nc.vector.tensor_tensor(t1, zB[0][:, s], WT[0][:, s],
                                        op=mybir.AluOpType.mult)
                nc.vector.tensor_tensor(t2, zB[1][:, s], WT[1][:, s],
                                        op=mybir.AluOpType.mult)
                nc.vector.tensor_tensor(VV[0][:, s], t1, t2,
                                        op=mybir.AluOpType.subtract)
                t3 = tp.tile([128, 2048], bf16, name="t3", tag="t1")
                t4 = tp.tile([128, 2048], bf16, name="t4", tag="t2")
                nc.vector.tensor_tensor(t3, zB[0][:, s], WT[1][:, s],
                                        op=mybir.AluOpType.mult)
                nc.vector.tensor_tensor(t4, zB[1][:, s], WT[0][:, s],
                                        op=mybir.AluOpType.mult)
                nc.vector.tensor_tensor(VV[1][:, s], t3, t4,
                                        op=mybir.AluOpType.add)

            fused_stage(zA, VV, PR_BDq, INV_BD_I)                 # S4
            fused_stage(zB, zA, PR_F2, INV_F2_I, stat="stride32")  # S5
            std_stage(yst, zB, (S_F2R, S_F2I, S_F2In))            # S6

            for t in range(4):
                for comp, dst in ((0, y0_d), (1, y1_d)):
                    nc.sync.dma_start(
                        out=dst.ap()[:, 8 * t:8 * (t + 1), :],
                        in_=yst[comp][:, 1024 * t:1024 * (t + 1)].rearrange(
                            "p (a c) -> p a c", a=8))
    return nc


_CACHE = {}


def _get_program():
    if "nc" not in _CACHE:
        nc = _build_program()
        try:
            if not nc.is_finalized():
                nc.finalize()
        except AttributeError:
            nc.finalize()
        _CACHE["nc"] = nc
    return _CACHE["nc"]


def _run(x, w_real, **kw):
    from concourse.bass_utils import run_bass_kernel_spmd

    nc = _get_program()
    consts = _consts_np()
    # host-side: transpose to (d2, d1, d3) and round to bf16
    xT = np.ascontiguousarray(
        np.asarray(x, dtype=np.float32).transpose(0, 2, 1, 3)).astype(BF)
    wT = np.asarray(w_real, dtype=np.float32).transpose(1, 0, 2)
    d2 = np.arange(D2)
    in_maps = []
    for c in range(NCORES):
        if SHARD_W:
            mod = np.exp(-2j * np.pi * (16 * c) * d2 / 128)
            wm = wT * mod[:, None, None]
            w0 = np.ascontiguousarray(wm.real, dtype=np.float32).astype(BF)
            w1 = np.ascontiguousarray(wm.imag, dtype=np.float32).astype(BF)
        else:
            w0 = np.ascontiguousarray(wT, dtype=np.float32).astype(BF)
            w1 = np.zeros_like(w0)
        in_maps.append({
            "x0": xT[2 * c],
            "x1": xT[2 * c + 1],
            "w0": w0,
            "w1": w1,
            "consts": consts,
        })
    res = run_bass_kernel_spmd(nc, in_maps, core_ids=list(range(NCORES)), **kw)
    out = np.empty((B, D1, D2, D3), dtype=np.float32)
    for c in range(NCORES):
        out[2 * c] = res.results[c]["y0"].transpose(1, 0, 2)
        out[2 * c + 1] = res.results[c]["y1"].transpose(1, 0, 2)
    return out, res


def kernel(x: np.ndarray, w_real: np.ndarray) -> np.ndarray:
    return _run(x, w_real)[0]


def kernel_traced(x: np.ndarray, w_real: np.ndarray):
    return _run(x, w_real, trace=True)


# revision 13
# speedup vs baseline: 1.5081x; 1.1241x over previous
"""Bass/Trainium2 kernel for batched 3D FFT circular convolution.

Reference computes y = Re(IFFT3(FFT3(x) * FFT3(w))) with net scaling
circular_conv3d(x, w) / sqrt(N); x: (16, 32, 128, 128) f32, w: (32, 128, 128).

Strategy (data parallel over batch, 8 cores x 2 samples):
- Pack two real samples as one complex volume z = x0 + i*x1; y0 = Re, y1 = Im.
- All FFTs as DFT matmuls in bf16 (inputs pre-rounded to bf16 on host).
- Transposes are FUSED into the DFT matmuls: the DFT matrix is symmetric, so
  making the DATA the stationary operand computes data^T @ F = (F @ data)^T --
  the transform output lands transposed (next axis on partitions) for free.
- Complex arithmetic pairs both component matrices in ONE 256-wide moving
  operand [F_R | F_I]; each data block needs only 2 matmuls (2 LDW).
- Layout permutations ride in strided stationary APs (single-strided slices,
  plus 32-col tile_position strips for stage S2); PSUM evictions contiguous.
- Host pre-transposes x,w to (d2,d1,d3) and un-transposes y so every DMA run
  is >= 1KB contiguous (otherwise DMA is descriptor-rate bound).
- W~ = FFT3(w)/(N*sqrt(N)) computed on-device per core (replicated; an
  AllGather-sharded variant measured WORSE: cross-core start skew ~20us makes
  any mid-kernel collective sync a net loss).
- W-chain stage groups are interleaved one stage ahead of the z-chain so the
  PE has independent work at the z-chain's all-to-all stage boundaries.

Stage layouts (partition | free):
  L0   [d2 | d1,d3]                  f = d1*128 + d3
  S1   fused FFT d2  -> [d3 | d1,k2] f = d1*128 + k2
  S2   fused FFT d3 (4 col-strips per block j, strided 32-col stationary)
                     -> [(k2q,d1) | j,k3]  f = j*128 + k3   (k2 = 4j+k2q)
  S3   BD FFT d1 (weight-stationary) -> [(k2q,k1) | j,k3]
  M    V = Z * W~   (DVE, bf16 2x mode)
  S4   fused BD IFFT d1 -> [k3 | 32*k2 + d1]
  S5   fused IFFT k3 (single-strided stationary, stride 32)
                     -> [k2 | d1,d3]  f = d1*128 + d3
  S6   IFFT k2 (weight-stationary) -> [d2 | d1,d3] -> DMA out
"""

import numpy as np
import ml_dtypes

BF = ml_dtypes.bfloat16

D1, D2, D3 = 32, 128, 128
NTOT = D1 * D2 * D3
FREE = D1 * D3  # 4096
B = 16
NCORES = 8

# paired (128x256) const slots
PF_F2, PI_F2, PF_F2s, PR_F2, PR_BDq, INV_F2_I, INV_BD_I = range(7)
NPAIRS_TOT = 7
# single 128-wide consts for weight-stationary stages
S_F2R, S_F2I, S_F2In, S_BDR, S_BDI, S_BDIn = range(6)
NSNG = 6


def _consts_np():
    k = np.arange(128)
    F2 = np.exp(-2j * np.pi * np.outer(k, k) / 128)
    k1 = np.arange(32)
    F1 = np.exp(-2j * np.pi * np.outer(k1, k1) / 32)
    BD = np.zeros((128, 128), complex)
    for g in range(4):
        BD[32 * g:32 * g + 32, 32 * g:32 * g + 32] = F1
    alpha = 1.0 / (NTOT * np.sqrt(np.float64(NTOT)))
    F2R, F2I = F2.real, F2.imag
    BDR, BDI = BD.real, BD.imag
    pairs = [
        np.concatenate([F2R, F2I], axis=1),             # PF_F2   (fwd, sR)
        np.concatenate([-F2I, F2R], axis=1),            # PI_F2   (fwd, sI)
        np.concatenate([F2R * alpha, F2I * alpha], 1),  # PF_F2s  (fwd, sR)
        np.concatenate([F2R, -F2I], axis=1),            # PR_F2   (inv, sR)
        np.concatenate([BDR, -BDI], axis=1),            # PR_BDq  (inv, sR)
        np.concatenate([F2I, F2R], axis=1),             # INV_F2_I (inv, sI)
        np.concatenate([BDI, BDR], axis=1),             # INV_BD_I (inv, sI)
    ]
    singles = [F2R, F2I, -F2I, BDR, BDI, -BDI]
    mats = np.concatenate([np.concatenate(pairs, axis=1),
                           np.concatenate(singles, axis=1)], axis=1)
    return np.ascontiguousarray(mats, dtype=np.float32).astype(BF)


def _build_program():
    import concourse.mybir as mybir
    import concourse.tile as tile
    from concourse import bacc

    f32 = mybir.dt.float32
    bf16 = mybir.dt.bfloat16

    nc = bacc.Bacc("TRN2")
    # inputs pre-transposed on host to (d2, d1, d3)
    x0_d = nc.dram_tensor("x0", (D2, D1, D3), bf16, kind="ExternalInput")
    x1_d = nc.dram_tensor("x1", (D2, D1, D3), bf16, kind="ExternalInput")
    w_d = nc.dram_tensor("w", (D2, D1, D3), bf16, kind="ExternalInput")
    CW = NPAIRS_TOT * 256 + NSNG * 128
    c_d = nc.dram_tensor("consts", (128, CW), bf16, kind="ExternalInput")
    y0_d = nc.dram_tensor("y0", (D2, D1, D3), f32, kind="ExternalOutput")
    y1_d = nc.dram_tensor("y1", (D2, D1, D3), f32, kind="ExternalOutput")

    with tile.TileContext(nc) as tc:
        with (
            tc.tile_pool(name="sb", bufs=1) as sb,
            tc.tile_pool(name="tp", bufs=2) as tp,
            tc.tile_pool(name="ps", bufs=2, space="PSUM") as ps,
        ):
            consts = sb.tile([128, CW], bf16, name="consts")
            nc.sync.dma_start(out=consts, in_=c_d.ap())

            def P2(i):
                return consts[:, 256 * i:256 * (i + 1)]

            def S1m(i):
                o = NPAIRS_TOT * 256
                return consts[:, o + 128 * i:o + 128 * (i + 1)]

            def vol(name, n=2, dt=bf16, cols=FREE):
                return [sb.tile([128, cols], dt, name=f"{name}{c}")
                        for c in range(n)]

            zA = vol("zA")
            zB = vol("zB")
            VV = vol("VV")
            wA = vol("wA", 1)
            wB = vol("wB")
            wC = vol("wC")
            WT = vol("WT")
            yst = vol("yst", 2, f32)

            # w first (gates the W chain = earliest PE work), then x halves
            nc.sync.dma_start(
                out=wA[0].rearrange("p (a c) -> p a c", a=D1),
                in_=w_d.ap())
            for t in range(2):
                for comp, src in ((0, x0_d), (1, x1_d)):
                    nc.sync.dma_start(
                        out=zA[comp][:, 2048 * t:2048 * (t + 1)].rearrange(
                            "p (a c) -> p a c", a=16),
                        in_=src.ap()[:, 16 * t:16 * (t + 1), :])

            ectr = [0]

            def evict(dst, src):
                # DVE also runs the ~18us multiply; give ACT ~7/12 of copies
                if (ectr[0] * 5) % 12 < 5:
                    nc.vector.tensor_copy(dst, src)
                else:
                    nc.scalar.copy(dst, src)
                ectr[0] += 1

            def lhs_for(src, b, stat):
                if stat == "contig":
                    return src[:, 128 * b:128 * (b + 1)]
                # "stride32": f = 32*k2 + d1 -> fixed d1=b, k2 stride 32
                v = src.rearrange("p (k2 d1) -> p k2 d1", k2=128, d1=32)
                return v[:, :, b:b + 1]

            def fused_group(dsts, srcs, pairR, pairI, g, stat="contig",
                            real_in=False):
                """one 8-block psum group of a fused stage."""
                pt = ps.tile([128, 2048], f32, name="pt", tag="ps")
                for q in range(8):
                    b = 8 * g + q
                    o = slice(256 * q, 256 * (q + 1))
                    st = (q % 2 == 0)
                    sp = (q % 2 == 1)
                    if stat == "strips":
                        v0 = srcs[0].rearrange("p (d1 k2) -> p k2 d1",
                                               d1=32, k2=128)
                        v1 = srcs[1].rearrange("p (d1 k2) -> p k2 d1",
                                               d1=32, k2=128)
                        for s in range(4):
                            po = pt[32 * s:32 * (s + 1), o]
                            nc.tensor.matmul(
                                po, v0[:, 4 * b + s, :], P2(pairR),
                                start=st, stop=False,
                                tile_position=(0, 32 * s),
                                skip_group_check=True)
                            nc.tensor.matmul(
                                po, v1[:, 4 * b + s, :], P2(pairI),
                                start=False, stop=sp,
                                tile_position=(0, 32 * s),
                                skip_group_check=True)
                    elif real_in:
                        nc.tensor.matmul(pt[:, o], lhs_for(srcs[0], b, stat),
                                         P2(pairR), start=st, stop=sp,
                                         skip_group_check=True)
                    else:
                        nc.tensor.matmul(pt[:, o], lhs_for(srcs[0], b, stat),
                                         P2(pairR), start=st, stop=False,
                                         skip_group_check=True)
                        nc.tensor.matmul(pt[:, o], lhs_for(srcs[1], b, stat),
                                         P2(pairI), start=False, stop=sp,
                                         skip_group_check=True)
                pv = pt.rearrange("p (q c f) -> p c q f", q=8, c=2)
                sl = slice(1024 * g, 1024 * (g + 1))
                dv0 = dsts[0][:, sl].rearrange("p (q f) -> p q f", q=8)
                dv1 = dsts[1][:, sl].rearrange("p (q f) -> p q f", q=8)
                evict(dv0, pv[:, 0])
                evict(dv1, pv[:, 1])

            def std_group(dsts, srcs, mats, t):
                """one 2-chunk (1024-col) psum group of a std stage."""
                mA, mB, mC = mats
                pt = ps.tile([128, 2048], f32, name="pt", tag="ps")
                for h in range(2):
                    s = slice(1024 * t + 512 * h, 1024 * t + 512 * (h + 1))
                    oR = slice(512 * h, 512 * (h + 1))
                    oI = slice(1024 + 512 * h, 1024 + 512 * (h + 1))
                    nc.tensor.matmul(pt[:, oR], S1m(mA), srcs[0][:, s],
                                     start=True, stop=False)
                    nc.tensor.matmul(pt[:, oI], S1m(mC), srcs[0][:, s],
                                     start=True, stop=False)
                    nc.tensor.matmul(pt[:, oR], S1m(mB), srcs[1][:, s],
                                     start=False, stop=True)
                    nc.tensor.matmul(pt[:, oI], S1m(mA), srcs[1][:, s],
                                     start=False, stop=True)
                sl = slice(1024 * t, 1024 * (t + 1))
                evict(dsts[0][:, sl], pt[:, :1024])
                evict(dsts[1][:, sl], pt[:, 1024:])

            FWD_BD = (S_BDR, S_BDIn, S_BDI)
            INV_F2s = (S_F2R, S_F2I, S_F2In)

            def mult_half(hh):
                s = slice(2048 * hh, 2048 * (hh + 1))
                t1 = tp.tile([128, 2048], bf16, name="t1", tag="t1")
                t2 = tp.tile([128, 2048], bf16, name="t2", tag="t2")
                nc.vector.tensor_tensor(t1, zB[0][:, s], WT[0][:, s],
                                        op=mybir.AluOpType.mult)
                nc.vector.tensor_tensor(t2, zB[1][:, s], WT[1][:, s],
                                        op=mybir.AluOpType.mult)
                nc.vector.tensor_tensor(VV[0][:, s], t1, t2,
                                        op=mybir.AluOpType.subtract)
                t3 = tp.tile([128, 2048], bf16, name="t3", tag="t1")
                t4 = tp.tile([128, 2048], bf16, name="t4", tag="t2")
                nc.vector.tensor_tensor(t3, zB[0][:, s], WT[1][:, s],
                                        op=mybir.AluOpType.mult)
                nc.vector.tensor_tensor(t4, zB[1][:, s], WT[0][:, s],
                                        op=mybir.AluOpType.mult)
                nc.vector.tensor_tensor(VV[1][:, s], t3, t4,
                                        op=mybir.AluOpType.add)

            # ---- schedule: W chain one stage ahead, interleaved with z ----
            for g in range(4):
                fused_group(wB, wA, PF_F2s, None, g, real_in=True)   # S1w
            for g in range(4):
                fused_group(zB, zA, PF_F2, PI_F2, g)                 # S1 g
                fused_group(wC, wB, PF_F2, PI_F2, g, stat="strips")  # S2w g
            for g in range(4):
                std_group(WT, wC, FWD_BD, g)                         # S3w g
                fused_group(zA, zB, PF_F2, PI_F2, g, stat="strips")  # S2 g
            for g in range(2):
                std_group(zB, zA, FWD_BD, 2 * g)                     # S3
                std_group(zB, zA, FWD_BD, 2 * g + 1)
                mult_half(g)                                         # M
            for g in range(4):
                fused_group(zA, VV, PR_BDq, INV_BD_I, g)             # S4
            for g in range(4):
                fused_group(zB, zA, PR_F2, INV_F2_I, g, stat="stride32")  # S5
            for g in range(4):
                std_group(yst, zB, INV_F2s, g)                       # S6
                for comp, dst in ((0, y0_d), (1, y1_d)):
                    nc.sync.dma_start(
                        out=dst.ap()[:, 8 * g:8 * (g + 1), :],
                        in_=yst[comp][:, 1024 * g:1024 * (g + 1)].rearrange(
                            "p (a c) -> p a c", a=8))
    return nc


_CACHE = {}


def _get_program():
    if "nc" not in _CACHE:
        nc = _build_program()
        try:
            if not nc.is_finalized():
                nc.finalize()
        except AttributeError:
            nc.finalize()
        _CACHE["nc"] = nc
    return _CACHE["nc"]


def _run(x, w_real, **kw):
    from concourse.bass_utils import run_bass_kernel_spmd

    nc = _get_program()
    consts = _consts_np()
    xT = np.ascontiguousarray(
        np.asarray(x, dtype=np.float32).transpose(0, 2, 1, 3)).astype(BF)
    wT = np.ascontiguousarray(
        np.asarray(w_real, dtype=np.float32).transpose(1, 0, 2)).astype(BF)
    in_maps = []
    for c in range(NCORES):
        in_maps.append({
            "x0": xT[2 * c],
            "x1": xT[2 * c + 1],
            "w": wT,
            "consts": consts,
        })
    res = run_bass_kernel_spmd(nc, in_maps, core_ids=list(range(NCORES)), **kw)
    out = np.empty((B, D1, D2, D3), dtype=np.float32)
    for c in range(NCORES):
        out[2 * c] = res.results[c]["y0"].transpose(1, 0, 2)
        out[2 * c + 1] = res.results[c]["y1"].transpose(1, 0, 2)
    return out, res


def kernel(x: np.ndarray, w_real: np.ndarray) -> np.ndarray:
    return _run(x, w_real)[0]


def kernel_traced(x: np.ndarray, w_real: np.ndarray):
    return _run(x, w_real, trace=True)
